# revision 29
# baseline (speedup 1.0000x reference)
"""LongcatMoe Trainium2 kernel — expert-parallel sparse MoE across 8 NeuronCores.

Strategy (expert-parallel, fp8 DoubleRow):
  - Host computes the tiny router (fp64 softmax/top-k) and dispatches tokens
    by top-k expert id: core e receives the tokens routed to expert e
    (capacity C=256; the few overflow tokens fall back to an exact host
    computation), plus expert e's weights quantized to fp8 e4m3.
  - Each core runs the silu-gated MLP for its expert with fp8 DoubleRow
    matmuls (2x PE throughput, contraction 256 per instruction):
      pg = sum_m (64 Wg)^T x        [I-tile, C] psum, = 64 g
      sg = silu(pg / 64)            ACT engine
      mid = (pu/16) * sg  -> fp8    DVE scalar_tensor_tensor, = 4 mid
      pd = sum_j (64 Wd)^T mid      = 256 d
      y  = bf16(pd)                 DMA out, host divides by 256
  - Host combines: out[tok] += (gate_w/256) * y, plus the zero-expert
    (identity) term zero_w[t] * x[t], both in fp64.

Scales: weights x64 (std 0.02 -> 1.28 keeps e4m3 normals), x unscaled,
mid x4 (max |4 mid| ~ 47 << 240 = e4m3 max). All scale factors are exact
powers of two and are undone in the host combine.

SBUF layouts (per-partition contiguous DMAs, >=2KB rows to keep the DMA
engines descriptor-efficient; few dma_starts since each trigger costs
~730ns of sequencer time; all input triggers ride ONE ring in consumption
order so the early critical tensors never compete for HBM bandwidth):
  xT  [128, HO, C]            xT[p, ho, t] = q(x[idx[t], ho*128+p]) (2 DMAs)
  wgm [HO/2, 128, 2, 2, IO, 128]  gate/up by h-tile-pair, 4KB rows (8 DMAs)
  wd2 [2, 128, 8, IO, 128]    down tiles in halves, 8KB rows (2 DMAs)
  y2  [8, 128, 2, C] bf16 output pairs (= 256 * down), 1KB rows (8 DMAs)

Phase 1 runs m-major (h-pair outer, i-tile inner) over 16 concurrent PSUM
accumulators so the PE starts as soon as wgm[0]+xT land, ~2.5us after the
DMA kick; the last h-pair is j-ordered so silu/mid production pipelines
into phase 2.
"""

import os

import numpy as np
import ml_dtypes

T, H, I, E, Z, TOPK = 1024, 2048, 1024, 8, 8, 4
ROUTED_SCALING = 1.0
N_CORES = 8
P = 128
HO = H // P  # 16
IO = I // P  # 8
C = 256      # per-expert device capacity; overflow handled on host
SW = 64.0    # weight quantization scale
SM = 4.0     # mid quantization scale

_PROGRAM = None
LAST_RESULTS = None  # BassKernelResults of the most recent run (for test harness)
ACT_FUNC = "Silu"   # overridden to "Sigmoid" by the CoreSim test (no Silu there)


def _build_program():
    import concourse.mybir as mybir
    import concourse.tile as tile
    from concourse import bacc

    f32 = mybir.dt.float32
    bf16 = mybir.dt.bfloat16
    fp8 = mybir.dt.float8e4
    SILU = getattr(mybir.ActivationFunctionType, ACT_FUNC)
    DR = mybir.MatmulPerfMode.DoubleRow
    MUL = mybir.AluOpType.mult

    nc = bacc.Bacc(
        "TRN2",
        target_bir_lowering=False,
        debug=False,
        enable_asserts=False,
        num_devices=N_CORES,
    )
    COPY = mybir.ActivationFunctionType.Copy
    xT = nc.dram_tensor("xT", [P, HO, C], fp8, kind="ExternalInput").ap()
    wgu = nc.dram_tensor("wgu", [IO, P, 2, HO, P], fp8,
                         kind="ExternalInput").ap()
    wd4 = nc.dram_tensor("wd4", [4, P, 4, IO, P], fp8,
                         kind="ExternalInput").ap()
    y2 = nc.dram_tensor("y2", [HO // 2, P, 2, C], bf16,
                        kind="ExternalOutput").ap()

    with tile.TileContext(nc) as tc:
        with (
            tc.tile_pool(name="px", bufs=1) as px,
            tc.tile_pool(name="pwg", bufs=IO) as pwg,
            tc.tile_pool(name="pwd", bufs=2) as pwd,
            tc.tile_pool(name="pmid", bufs=1) as pmid,
            tc.tile_pool(name="psg", bufs=2) as psg,
            tc.tile_pool(name="py", bufs=4) as py,
            tc.tile_pool(name="pwrm", bufs=1) as pwrm,
            tc.tile_pool(name="ppg", bufs=2, space="PSUM") as ppg,
            tc.tile_pool(name="ppu", bufs=2, space="PSUM") as ppu,
            tc.tile_pool(name="ppd", bufs=3, space="PSUM") as ppd,
            tc.tile_pool(name="ppw", bufs=1, space="PSUM") as ppw,
        ):
            # PE warmup: keep the tensor engine busy (and its clock ramping)
            # while the first input DMAs land.
            wtile = pwrm.tile([P, 512], bf16)
            nc.vector.memset(wtile[:], 0.0)
            pwm = ppw.tile([P, 512], f32)
            for w in range(12):
                nc.tensor.matmul(pwm[:], wtile[:, :P], wtile[:],
                                 start=(w == 0), stop=(w == 11))

            # All input DMAs ride the SP ring in strict consumption order
            # (weights before wd): the queues then serve the critical head
            # tensors with the full HBM bandwidth. wgu[0] is split so the
            # first gate matmuls can start before its up-half lands.
            xt = px.tile([P, HO, C], fp8)
            wgu_t = [pwg.tile([P, 2, HO, P], fp8, name=f"wgu{j}", tag="wgu")
                     for j in range(IO)]
            wd_t = [pwd.tile([P, 4, IO, P], fp8, name=f"wd{h}", tag="wd")
                    for h in range(4)]

            XC = HO // 2  # xt in 2 chunks
            nc.sync.dma_start(wgu_t[0][:, 0], wgu[0, :, 0])
            nc.sync.dma_start(xt[:, 0:XC, :], xT[:, 0:XC, :])
            nc.sync.dma_start(xt[:, XC:HO, :], xT[:, XC:HO, :])
            nc.sync.dma_start(wgu_t[0][:, 1], wgu[0, :, 1])
            for j in range(1, IO):
                nc.sync.dma_start(wgu_t[j][:], wgu[j])
            for h in range(4):
                nc.sync.dma_start(wd_t[h][:], wd4[h])

            # Phase 1, j-major: pg/pu in separate PSUM banks with proper
            # start/stop accumulation groups (hardware PSUM zeroing is
            # bank-granular; one live group per bank).
            mid = pmid.tile([P, IO, C], fp8)
            for j in range(IO):
                pg = ppg.tile([P, C], f32)
                pu = ppu.tile([P, C], f32)
                for m in range(HO // 2):
                    nc.tensor.matmul(
                        pg[:], wgu_t[j][:, 0, 2 * m:2 * m + 2, :],
                        xt[:, 2 * m:2 * m + 2, :],
                        start=(m == 0), stop=(m == HO // 2 - 1),
                        perf_mode=DR,
                    )
                for m in range(HO // 2):
                    nc.tensor.matmul(
                        pu[:], wgu_t[j][:, 1, 2 * m:2 * m + 2, :],
                        xt[:, 2 * m:2 * m + 2, :],
                        start=(m == 0), stop=(m == HO // 2 - 1),
                        perf_mode=DR,
                    )
                sg = psg.tile([P, C], f32)
                nc.scalar.activation(sg[:], pg[:], SILU, scale=1.0 / SW)
                nc.vector.scalar_tensor_tensor(
                    out=mid[:, j, :], in0=pu[:], scalar=SM / SW, in1=sg[:],
                    op0=MUL, op1=MUL,
                )

            # Phase 2: y[k] = sum_j Wd[j, k].T @ mid[j] (i-tile pairs),
            # emitted in pairs of h-tiles per output DMA. pd banks rotate
            # through all three PSUM pools (7 banks) so bank recycling never
            # paces the PE; the PSUM->SBUF casts alternate between the ACT
            # and DVE engines and the output DMA triggers alternate between
            # the two otherwise-idle rings (GpSimd, SP).
            pd_pools = [(ppd, "pd"), (ppg, "pg"), (ppu, "pu")]
            for q in range(HO // 2):
                ty = py.tile([P, 2, C], bf16)
                for kk in range(2):
                    k = 2 * q + kk
                    pool, pnm = pd_pools[k % 3]
                    pd = pool.tile([P, C], f32, name=pnm)
                    for j in range(IO // 2):
                        nc.tensor.matmul(
                            pd[:], wd_t[k // 4][:, k % 4, 2 * j:2 * j + 2, :],
                            mid[:, 2 * j:2 * j + 2, :],
                            start=(j == 0), stop=(j == IO // 2 - 1),
                            perf_mode=DR,
                        )
                    if kk == 0:
                        nc.scalar.activation(ty[:, kk, :], pd[:], COPY)
                    else:
                        nc.vector.tensor_copy(out=ty[:, kk, :], in_=pd[:])
                if q % 2 == 0:
                    nc.gpsimd.dma_start(y2[q], ty[:])
                else:
                    nc.sync.dma_start(y2[q], ty[:])

    nc.compile()
    return nc


def _route(x, router_w, corr_bias):
    """fp64 router: returns (topk_idx [T,K], topk_w [T,K])."""
    xl = x.astype(np.float64)
    logits = xl @ router_w.astype(np.float64).T
    logits -= logits.max(axis=1, keepdims=True)
    p = np.exp(logits)
    p /= p.sum(axis=1, keepdims=True)
    sel = p + corr_bias.astype(np.float64)
    topk_idx = np.argsort(-sel, axis=1, kind="stable")[:, :TOPK]
    topk_w = np.take_along_axis(p, topk_idx, axis=1) * ROUTED_SCALING
    return topk_idx, topk_w


def _pack_inputs(x8_toks, wg_e, wu_e, wd_e):
    """Device-layout packing for one expert: x8_toks [n<=C, H] fp8."""
    f8 = ml_dtypes.float8_e4m3

    def q8(a):
        return np.clip(a, -240.0, 240.0).astype(f8)

    n = len(x8_toks)
    xg = np.zeros((C, H), dtype=f8)
    xg[:n] = x8_toks
    xTd = np.ascontiguousarray(xg.T.reshape(HO, P, C).transpose(1, 0, 2))
    wgd = q8(SW * wg_e).reshape(HO, P, IO, P).transpose(2, 1, 0, 3)
    wud = q8(SW * wu_e).reshape(HO, P, IO, P).transpose(2, 1, 0, 3)
    wgud = np.ascontiguousarray(np.stack([wgd, wud], axis=2))  # [IO,P,2,HO,P]
    wdd = np.ascontiguousarray(
        q8(SW * wd_e).reshape(IO, P, HO, P)
        .transpose(2, 1, 0, 3)                 # [HO, P, IO, P]
        .reshape(4, 4, P, IO, P)
        .transpose(0, 2, 1, 3, 4))             # [4, P, 4, IO, P]
    return {"xT": xTd, "wgu": wgud, "wd4": wdd}


def kernel(hidden_states, router_w, corr_bias, w_gate, w_up, w_down):
    global _PROGRAM, LAST_RESULTS
    x = np.asarray(hidden_states, dtype=np.float32)
    router_w = np.asarray(router_w, dtype=np.float32)
    corr_bias = np.asarray(corr_bias, dtype=np.float32)
    w_gate = np.asarray(w_gate, dtype=np.float32)
    w_up = np.asarray(w_up, dtype=np.float32)
    w_down = np.asarray(w_down, dtype=np.float32)

    topk_idx, topk_w = _route(x, router_w, corr_bias)
    routed = topk_idx < E
    zero_w = (topk_w * (~routed)).sum(axis=1)  # [T] fp64

    f8 = ml_dtypes.float8_e4m3  # TRN fp8e4: e4m3 with max normal 240

    def q8(a):
        return np.clip(a, -240.0, 240.0).astype(f8)

    x8 = q8(x)

    # Dispatch: token list + gate weight per expert; overflow beyond C
    # falls back to an exact host computation.
    idx_list, w_list, overflow = [], [], []
    for e in range(E):
        toks, kpos = np.nonzero(topk_idx == e)
        we = topk_w[toks, kpos]
        if len(toks) > C:
            overflow.append((e, toks[C:], we[C:]))
            toks, we = toks[:C], we[:C]
        idx_list.append(toks)
        w_list.append(we)

    in_maps = [
        _pack_inputs(x8[idx_list[e]], w_gate[e], w_up[e], w_down[e])
        for e in range(E)
    ]

    if _PROGRAM is None:
        _PROGRAM = _build_program()

    from concourse.bass_utils import run_bass_kernel_spmd

    kw = {}
    if os.environ.get("MOE_KERNEL_TRACE", "") == "1":
        kw = dict(trace=True, trace_cores=list(range(N_CORES)))
    res = run_bass_kernel_spmd(
        _PROGRAM, in_maps, core_ids=list(range(N_CORES)), **kw)
    LAST_RESULTS = res

    out = np.zeros((T, H), dtype=np.float64)
    inv = 1.0 / (SW * SM)
    for e in range(E):
        n = len(idx_list[e])
        if n:
            y2e = res.results[e]["y2"]  # [HO/2, P, 2, C] bf16 = 256 * down
            ye = y2e.transpose(0, 2, 1, 3).reshape(H, C)
            out[idx_list[e]] += (w_list[e] * inv)[:, None] * \
                ye[:, :n].T.astype(np.float64)
    for e, toks, ws in overflow:
        xt = x[toks].astype(np.float64)
        g = xt @ w_gate[e].astype(np.float64)
        u = xt @ w_up[e].astype(np.float64)
        mid = (g / (1.0 + np.exp(-g))) * u
        out[toks] += ws[:, None] * (mid @ w_down[e].astype(np.float64))
    out += zero_w[:, None] * x.astype(np.float64)
    return out.astype(np.float32)


# revision 30
# speedup vs baseline: 1.0236x; 1.0236x over previous
"""LongcatMoe Trainium2 kernel — expert-parallel sparse MoE across 8 NeuronCores.

Strategy (expert-parallel, fp8 DoubleRow):
  - Host computes the tiny router (fp64 softmax/top-k) and dispatches tokens
    by top-k expert id: core e receives the tokens routed to expert e
    (capacity C=256; the few overflow tokens fall back to an exact host
    computation), plus expert e's weights quantized to fp8 e4m3.
  - Each core runs the silu-gated MLP for its expert with fp8 DoubleRow
    matmuls (2x PE throughput, contraction 256 per instruction):
      pg = sum_m (64 Wg)^T x        [I-tile, C] psum, = 64 g
      sg = silu(pg / 64)            ACT engine
      mid = (pu/16) * sg  -> fp8    DVE scalar_tensor_tensor, = 4 mid
      pd = sum_j (64 Wd)^T mid      = 256 d
      y  = bf16(pd)                 DMA out, host divides by 256
  - Host combines: out[tok] += (gate_w/256) * y, plus the zero-expert
    (identity) term zero_w[t] * x[t], both in fp64.

Scales: weights x64 (std 0.02 -> 1.28 keeps e4m3 normals), x unscaled,
mid x4 (max |4 mid| ~ 47 << 240 = e4m3 max). All scale factors are exact
powers of two and are undone in the host combine.

SBUF layouts (per-partition contiguous DMAs, >=2KB rows to keep the DMA
engines descriptor-efficient; few dma_starts since each trigger costs
~730ns of sequencer time; all input triggers ride ONE ring in consumption
order so the early critical tensors never compete for HBM bandwidth):
  xT  [128, HO, C]            xT[p, ho, t] = q(x[idx[t], ho*128+p]) (2 DMAs)
  wgm [HO/2, 128, 2, 2, IO, 128]  gate/up by h-tile-pair, 4KB rows (8 DMAs)
  wd2 [2, 128, 8, IO, 128]    down tiles in halves, 8KB rows (2 DMAs)
  y2  [8, 128, 2, C] bf16 output pairs (= 256 * down), 1KB rows (8 DMAs)

Phase 1 runs m-major (h-pair outer, i-tile inner) over 16 concurrent PSUM
accumulators so the PE starts as soon as wgm[0]+xT land, ~2.5us after the
DMA kick; the last h-pair is j-ordered so silu/mid production pipelines
into phase 2.
"""

import os

import numpy as np
import ml_dtypes

T, H, I, E, Z, TOPK = 1024, 2048, 1024, 8, 8, 4
ROUTED_SCALING = 1.0
N_CORES = 8
P = 128
HO = H // P  # 16
IO = I // P  # 8
C = 256      # per-expert device capacity; overflow handled on host
SW = 64.0    # weight quantization scale
SM = 4.0     # mid quantization scale

_PROGRAM = None
LAST_RESULTS = None  # BassKernelResults of the most recent run (for test harness)
ACT_FUNC = "Silu"   # overridden to "Sigmoid" by the CoreSim test (no Silu there)


def _build_program():
    import concourse.mybir as mybir
    import concourse.tile as tile
    from concourse import bacc

    f32 = mybir.dt.float32
    bf16 = mybir.dt.bfloat16
    fp8 = mybir.dt.float8e4
    SILU = getattr(mybir.ActivationFunctionType, ACT_FUNC)
    DR = mybir.MatmulPerfMode.DoubleRow
    MUL = mybir.AluOpType.mult

    nc = bacc.Bacc(
        "TRN2",
        target_bir_lowering=False,
        debug=False,
        enable_asserts=False,
        num_devices=N_CORES,
    )
    COPY = mybir.ActivationFunctionType.Copy
    xT = nc.dram_tensor("xT", [P, HO, C], fp8, kind="ExternalInput").ap()
    wgu = nc.dram_tensor("wgu", [IO, P, 2, HO, P], fp8,
                         kind="ExternalInput").ap()
    wd4 = nc.dram_tensor("wd4", [4, P, 4, IO, P], fp8,
                         kind="ExternalInput").ap()
    y2 = nc.dram_tensor("y2", [HO // 2, P, 2, C], bf16,
                        kind="ExternalOutput").ap()

    with tile.TileContext(nc) as tc:
        with (
            tc.tile_pool(name="px", bufs=1) as px,
            tc.tile_pool(name="pwg", bufs=IO) as pwg,
            tc.tile_pool(name="pwd", bufs=2) as pwd,
            tc.tile_pool(name="pmid", bufs=1) as pmid,
            tc.tile_pool(name="psg", bufs=2) as psg,
            tc.tile_pool(name="py", bufs=4) as py,
            tc.tile_pool(name="pwrm", bufs=1) as pwrm,
            tc.tile_pool(name="ppg", bufs=2, space="PSUM") as ppg,
            tc.tile_pool(name="ppu", bufs=2, space="PSUM") as ppu,
            tc.tile_pool(name="ppd", bufs=3, space="PSUM") as ppd,
            tc.tile_pool(name="ppw", bufs=1, space="PSUM") as ppw,
        ):
            # PE warmup: keep the tensor engine busy (and its clock ramping)
            # while the first input DMAs land.
            wtile = pwrm.tile([P, 512], bf16)
            nc.vector.memset(wtile[:], 0.0)
            pwm = ppw.tile([P, 512], f32)
            for w in range(7):
                nc.tensor.matmul(pwm[:], wtile[:, :P], wtile[:],
                                 start=(w == 0), stop=(w == 6))

            # All input DMAs ride the SP ring in strict consumption order
            # (weights before wd): the queues then serve the critical head
            # tensors with the full HBM bandwidth. wgu[0] is split so the
            # first gate matmuls can start before its up-half lands.
            xt = px.tile([P, HO, C], fp8)
            wgu_t = [pwg.tile([P, 2, HO, P], fp8, name=f"wgu{j}", tag="wgu")
                     for j in range(IO)]
            wd_t = [pwd.tile([P, 4, IO, P], fp8, name=f"wd{h}", tag="wd")
                    for h in range(4)]

            XC = HO // 2  # xt in 2 chunks
            nc.sync.dma_start(wgu_t[0][:, 0], wgu[0, :, 0])
            nc.sync.dma_start(xt[:, 0:XC, :], xT[:, 0:XC, :])
            nc.sync.dma_start(xt[:, XC:HO, :], xT[:, XC:HO, :])
            nc.sync.dma_start(wgu_t[0][:, 1], wgu[0, :, 1])
            for j in range(1, IO):
                nc.sync.dma_start(wgu_t[j][:], wgu[j])
                if j == 5:
                    nc.sync.dma_start(wd_t[0][:], wd4[0])
                elif j == 6:
                    nc.sync.dma_start(wd_t[1][:], wd4[1])
            for h in range(2, 4):
                nc.sync.dma_start(wd_t[h][:], wd4[h])

            # Phase 1, j-major: pg/pu in separate PSUM banks with proper
            # start/stop accumulation groups (hardware PSUM zeroing is
            # bank-granular; one live group per bank).
            mid = pmid.tile([P, IO, C], fp8)
            for j in range(IO):
                pg = ppg.tile([P, C], f32)
                pu = ppu.tile([P, C], f32)
                for m in range(HO // 2):
                    nc.tensor.matmul(
                        pg[:], wgu_t[j][:, 0, 2 * m:2 * m + 2, :],
                        xt[:, 2 * m:2 * m + 2, :],
                        start=(m == 0), stop=(m == HO // 2 - 1),
                        perf_mode=DR,
                    )
                for m in range(HO // 2):
                    nc.tensor.matmul(
                        pu[:], wgu_t[j][:, 1, 2 * m:2 * m + 2, :],
                        xt[:, 2 * m:2 * m + 2, :],
                        start=(m == 0), stop=(m == HO // 2 - 1),
                        perf_mode=DR,
                    )
                sg = psg.tile([P, C], f32)
                nc.scalar.activation(sg[:], pg[:], SILU, scale=1.0 / SW)
                nc.vector.scalar_tensor_tensor(
                    out=mid[:, j, :], in0=pu[:], scalar=SM / SW, in1=sg[:],
                    op0=MUL, op1=MUL,
                )

            # Phase 2: y[k] = sum_j Wd[j, k].T @ mid[j] (i-tile pairs),
            # emitted in pairs of h-tiles per output DMA. pd banks rotate
            # through all three PSUM pools (7 banks) so bank recycling never
            # paces the PE; the PSUM->SBUF casts alternate between the ACT
            # and DVE engines and the output DMA triggers alternate between
            # the two otherwise-idle rings (GpSimd, SP).
            pd_pools = [(ppd, "pd"), (ppg, "pg"), (ppu, "pu")]
            for q in range(HO // 2):
                ty = py.tile([P, 2, C], bf16)
                for kk in range(2):
                    k = 2 * q + kk
                    pool, pnm = pd_pools[k % 3]
                    pd = pool.tile([P, C], f32, name=pnm)
                    for j in range(IO // 2):
                        nc.tensor.matmul(
                            pd[:], wd_t[k // 4][:, k % 4, 2 * j:2 * j + 2, :],
                            mid[:, 2 * j:2 * j + 2, :],
                            start=(j == 0), stop=(j == IO // 2 - 1),
                            perf_mode=DR,
                        )
                    if kk == 0:
                        nc.scalar.activation(ty[:, kk, :], pd[:], COPY)
                    else:
                        nc.vector.tensor_copy(out=ty[:, kk, :], in_=pd[:])
                if q % 2 == 0:
                    nc.gpsimd.dma_start(y2[q], ty[:])
                else:
                    nc.sync.dma_start(y2[q], ty[:])

    nc.compile()
    return nc


def _route(x, router_w, corr_bias):
    """fp64 router: returns (topk_idx [T,K], topk_w [T,K])."""
    xl = x.astype(np.float64)
    logits = xl @ router_w.astype(np.float64).T
    logits -= logits.max(axis=1, keepdims=True)
    p = np.exp(logits)
    p /= p.sum(axis=1, keepdims=True)
    sel = p + corr_bias.astype(np.float64)
    topk_idx = np.argsort(-sel, axis=1, kind="stable")[:, :TOPK]
    topk_w = np.take_along_axis(p, topk_idx, axis=1) * ROUTED_SCALING
    return topk_idx, topk_w


def _pack_inputs(x8_toks, wg_e, wu_e, wd_e):
    """Device-layout packing for one expert: x8_toks [n<=C, H] fp8."""
    f8 = ml_dtypes.float8_e4m3

    def q8(a):
        return np.clip(a, -240.0, 240.0).astype(f8)

    n = len(x8_toks)
    xg = np.zeros((C, H), dtype=f8)
    xg[:n] = x8_toks
    xTd = np.ascontiguousarray(xg.T.reshape(HO, P, C).transpose(1, 0, 2))
    wgd = q8(SW * wg_e).reshape(HO, P, IO, P).transpose(2, 1, 0, 3)
    wud = q8(SW * wu_e).reshape(HO, P, IO, P).transpose(2, 1, 0, 3)
    wgud = np.ascontiguousarray(np.stack([wgd, wud], axis=2))  # [IO,P,2,HO,P]
    wdd = np.ascontiguousarray(
        q8(SW * wd_e).reshape(IO, P, HO, P)
        .transpose(2, 1, 0, 3)                 # [HO, P, IO, P]
        .reshape(4, 4, P, IO, P)
        .transpose(0, 2, 1, 3, 4))             # [4, P, 4, IO, P]
    return {"xT": xTd, "wgu": wgud, "wd4": wdd}


def kernel(hidden_states, router_w, corr_bias, w_gate, w_up, w_down):
    global _PROGRAM, LAST_RESULTS
    x = np.asarray(hidden_states, dtype=np.float32)
    router_w = np.asarray(router_w, dtype=np.float32)
    corr_bias = np.asarray(corr_bias, dtype=np.float32)
    w_gate = np.asarray(w_gate, dtype=np.float32)
    w_up = np.asarray(w_up, dtype=np.float32)
    w_down = np.asarray(w_down, dtype=np.float32)

    topk_idx, topk_w = _route(x, router_w, corr_bias)
    routed = topk_idx < E
    zero_w = (topk_w * (~routed)).sum(axis=1)  # [T] fp64

    f8 = ml_dtypes.float8_e4m3  # TRN fp8e4: e4m3 with max normal 240

    def q8(a):
        return np.clip(a, -240.0, 240.0).astype(f8)

    x8 = q8(x)

    # Dispatch: token list + gate weight per expert; overflow beyond C
    # falls back to an exact host computation.
    idx_list, w_list, overflow = [], [], []
    for e in range(E):
        toks, kpos = np.nonzero(topk_idx == e)
        we = topk_w[toks, kpos]
        if len(toks) > C:
            overflow.append((e, toks[C:], we[C:]))
            toks, we = toks[:C], we[:C]
        idx_list.append(toks)
        w_list.append(we)

    in_maps = [
        _pack_inputs(x8[idx_list[e]], w_gate[e], w_up[e], w_down[e])
        for e in range(E)
    ]

    if _PROGRAM is None:
        _PROGRAM = _build_program()

    from concourse.bass_utils import run_bass_kernel_spmd

    kw = {}
    if os.environ.get("MOE_KERNEL_TRACE", "") == "1":
        kw = dict(trace=True, trace_cores=list(range(N_CORES)))
    res = run_bass_kernel_spmd(
        _PROGRAM, in_maps, core_ids=list(range(N_CORES)), **kw)
    LAST_RESULTS = res

    out = np.zeros((T, H), dtype=np.float64)
    inv = 1.0 / (SW * SM)
    for e in range(E):
        n = len(idx_list[e])
        if n:
            y2e = res.results[e]["y2"]  # [HO/2, P, 2, C] bf16 = 256 * down
            ye = y2e.transpose(0, 2, 1, 3).reshape(H, C)
            out[idx_list[e]] += (w_list[e] * inv)[:, None] * \
                ye[:, :n].T.astype(np.float64)
    for e, toks, ws in overflow:
        xt = x[toks].astype(np.float64)
        g = xt @ w_gate[e].astype(np.float64)
        u = xt @ w_up[e].astype(np.float64)
        mid = (g / (1.0 + np.exp(-g))) * u
        out[toks] += ws[:, None] * (mid @ w_down[e].astype(np.float64))
    out += zero_w[:, None] * x.astype(np.float64)
    return out.astype(np.float32)


# revision 31
# speedup vs baseline: 1.2022x; 1.1745x over previous
"""LongcatMoe Trainium2 kernel — expert-parallel sparse MoE across 8 NeuronCores.

Strategy (expert-parallel, fp8 DoubleRow):
  - Host computes the tiny router (fp64 softmax/top-k) and dispatches tokens
    by top-k expert id: core e receives the tokens routed to expert e
    (capacity C=256; the few overflow tokens fall back to an exact host
    computation), plus expert e's weights quantized to fp8 e4m3.
  - Each core runs the silu-gated MLP for its expert with fp8 DoubleRow
    matmuls (2x PE throughput, contraction 256 per instruction):
      pg = sum_m (64 Wg)^T x        [I-tile, C] psum, = 64 g
      sg = silu(pg / 64)            ACT engine
      mid = (pu/16) * sg  -> fp8    DVE scalar_tensor_tensor, = 4 mid
      pd = sum_j (64 Wd)^T mid      = 256 d
      y  = bf16(pd)                 DMA out, host divides by 256
  - Host combines: out[tok] += (gate_w/256) * y, plus the zero-expert
    (identity) term zero_w[t] * x[t], both in fp64.

Scales: weights x64 (std 0.02 -> 1.28 keeps e4m3 normals), x unscaled,
mid x4 (max |4 mid| ~ 47 << 240 = e4m3 max). All scale factors are exact
powers of two and are undone in the host combine.

SBUF layouts (per-partition contiguous DMAs, >=2KB rows to keep the DMA
engines descriptor-efficient; few dma_starts since each trigger costs
~730ns of sequencer time; all input triggers ride ONE ring in consumption
order so the early critical tensors never compete for HBM bandwidth):
  xT  [128, HO, C]            xT[p, ho, t] = q(x[idx[t], ho*128+p]) (2 DMAs)
  wgm [HO/2, 128, 2, 2, IO, 128]  gate/up by h-tile-pair, 4KB rows (8 DMAs)
  wd2 [2, 128, 8, IO, 128]    down tiles in halves, 8KB rows (2 DMAs)
  y2  [8, 128, 2, C] bf16 output pairs (= 256 * down), 1KB rows (8 DMAs)

Phase 1 runs m-major (h-pair outer, i-tile inner) over 16 concurrent PSUM
accumulators so the PE starts as soon as wgm[0]+xT land, ~2.5us after the
DMA kick; the last h-pair is j-ordered so silu/mid production pipelines
into phase 2.
"""

import os

import numpy as np
import ml_dtypes

T, H, I, E, Z, TOPK = 1024, 2048, 1024, 8, 8, 4
ROUTED_SCALING = 1.0
N_CORES = 8
P = 128
HO = H // P  # 16
IO = I // P  # 8
C = 256      # per-expert device capacity; overflow handled on host
SW = 64.0    # weight quantization scale
SM = 4.0     # mid quantization scale

_PROGRAM = None
LAST_RESULTS = None  # BassKernelResults of the most recent run (for test harness)
ACT_FUNC = "Silu"   # overridden to "Sigmoid" by the CoreSim test (no Silu there)


def _build_program():
    import concourse.mybir as mybir
    import concourse.tile as tile
    from concourse import bacc

    f32 = mybir.dt.float32
    bf16 = mybir.dt.bfloat16
    fp8 = mybir.dt.float8e4
    SILU = getattr(mybir.ActivationFunctionType, ACT_FUNC)
    DR = mybir.MatmulPerfMode.DoubleRow
    MUL = mybir.AluOpType.mult

    nc = bacc.Bacc(
        "TRN2",
        target_bir_lowering=False,
        debug=False,
        enable_asserts=False,
        num_devices=N_CORES,
    )
    COPY = mybir.ActivationFunctionType.Copy
    xT = nc.dram_tensor("xT", [P, HO, C], fp8, kind="ExternalInput").ap()
    wgu = nc.dram_tensor("wgu", [IO, P, 2, HO, P], fp8,
                         kind="ExternalInput").ap()
    wd4 = nc.dram_tensor("wd4", [4, P, 4, IO, P], fp8,
                         kind="ExternalInput").ap()
    y2 = nc.dram_tensor("y2", [HO // 2, P, 2, C], bf16,
                        kind="ExternalOutput").ap()

    with tile.TileContext(nc) as tc:
        with (
            tc.tile_pool(name="px", bufs=1) as px,
            tc.tile_pool(name="pwg", bufs=IO) as pwg,
            tc.tile_pool(name="pwd", bufs=4) as pwd,
            tc.tile_pool(name="pmid", bufs=1) as pmid,
            tc.tile_pool(name="psg", bufs=2) as psg,
            tc.tile_pool(name="py", bufs=4) as py,
            tc.tile_pool(name="pwrm", bufs=1) as pwrm,
            tc.tile_pool(name="ppg", bufs=2, space="PSUM") as ppg,
            tc.tile_pool(name="ppu", bufs=2, space="PSUM") as ppu,
            tc.tile_pool(name="ppd", bufs=3, space="PSUM") as ppd,
            tc.tile_pool(name="ppw", bufs=1, space="PSUM") as ppw,
        ):
            # PE warmup: keep the tensor engine busy (and its clock ramping)
            # while the first input DMAs land.
            wtile = pwrm.tile([P, 512], bf16)
            nc.vector.memset(wtile[:], 0.0)
            pwm = ppw.tile([P, 512], f32)
            for w in range(7):
                nc.tensor.matmul(pwm[:], wtile[:, :P], wtile[:],
                                 start=(w == 0), stop=(w == 6))

            # All input DMAs ride the SP ring in strict consumption order
            # (weights before wd): the queues then serve the critical head
            # tensors with the full HBM bandwidth. wgu[0] is split so the
            # first gate matmuls can start before its up-half lands.
            xt = px.tile([P, HO, C], fp8)
            wgu_t = [pwg.tile([P, 2, HO, P], fp8, name=f"wgu{j}", tag="wgu")
                     for j in range(IO)]
            wd_t = [pwd.tile([P, 4, IO, P], fp8, name=f"wd{h}", tag="wd")
                    for h in range(4)]

            XC = HO // 2  # xt in 2 chunks
            nc.sync.dma_start(wgu_t[0][:, 0], wgu[0, :, 0])
            nc.sync.dma_start(xt[:, 0:XC, :], xT[:, 0:XC, :])
            nc.sync.dma_start(xt[:, XC:HO, :], xT[:, XC:HO, :])
            nc.sync.dma_start(wgu_t[0][:, 1], wgu[0, :, 1])
            for j in range(1, IO):
                nc.sync.dma_start(wgu_t[j][:], wgu[j])
                if j == 5:
                    nc.sync.dma_start(wd_t[0][:], wd4[0])
                elif j == 6:
                    nc.sync.dma_start(wd_t[1][:], wd4[1])
            for h in range(2, 4):
                nc.sync.dma_start(wd_t[h][:], wd4[h])

            # Phase 1, j-major: pg/pu in separate PSUM banks with proper
            # start/stop accumulation groups (hardware PSUM zeroing is
            # bank-granular; one live group per bank).
            mid = pmid.tile([P, IO, C], fp8)
            for j in range(IO):
                pg = ppg.tile([P, C], f32)
                pu = ppu.tile([P, C], f32)
                for m in range(HO // 2):
                    nc.tensor.matmul(
                        pg[:], wgu_t[j][:, 0, 2 * m:2 * m + 2, :],
                        xt[:, 2 * m:2 * m + 2, :],
                        start=(m == 0), stop=(m == HO // 2 - 1),
                        perf_mode=DR,
                    )
                for m in range(HO // 2):
                    nc.tensor.matmul(
                        pu[:], wgu_t[j][:, 1, 2 * m:2 * m + 2, :],
                        xt[:, 2 * m:2 * m + 2, :],
                        start=(m == 0), stop=(m == HO // 2 - 1),
                        perf_mode=DR,
                    )
                sg = psg.tile([P, C], f32)
                nc.scalar.activation(sg[:], pg[:], SILU, scale=1.0 / SW)
                nc.vector.scalar_tensor_tensor(
                    out=mid[:, j, :], in0=pu[:], scalar=SM / SW, in1=sg[:],
                    op0=MUL, op1=MUL,
                )

            # Phase 2: y[k] = sum_j Wd[j, k].T @ mid[j] (i-tile pairs),
            # emitted in pairs of h-tiles per output DMA. pd banks rotate
            # through all three PSUM pools (7 banks) so bank recycling never
            # paces the PE; the PSUM->SBUF casts alternate between the ACT
            # and DVE engines and the output DMA triggers alternate between
            # the two otherwise-idle rings (GpSimd, SP).
            pd_pools = [(ppd, "pd"), (ppg, "pg"), (ppu, "pu")]
            for q in range(HO // 2):
                ty = py.tile([P, 2, C], bf16)
                for kk in range(2):
                    k = 2 * q + kk
                    pool, pnm = pd_pools[k % 3]
                    pd = pool.tile([P, C], f32, name=pnm)
                    for j in range(IO // 2):
                        nc.tensor.matmul(
                            pd[:], wd_t[k // 4][:, k % 4, 2 * j:2 * j + 2, :],
                            mid[:, 2 * j:2 * j + 2, :],
                            start=(j == 0), stop=(j == IO // 2 - 1),
                            perf_mode=DR,
                        )
                    if kk == 0:
                        nc.scalar.activation(ty[:, kk, :], pd[:], COPY)
                    else:
                        nc.vector.tensor_copy(out=ty[:, kk, :], in_=pd[:])
                if q % 2 == 0:
                    nc.gpsimd.dma_start(y2[q], ty[:])
                else:
                    nc.sync.dma_start(y2[q], ty[:])

    nc.compile()
    return nc


def _route(x, router_w, corr_bias):
    """fp64 router: returns (topk_idx [T,K], topk_w [T,K])."""
    xl = x.astype(np.float64)
    logits = xl @ router_w.astype(np.float64).T
    logits -= logits.max(axis=1, keepdims=True)
    p = np.exp(logits)
    p /= p.sum(axis=1, keepdims=True)
    sel = p + corr_bias.astype(np.float64)
    topk_idx = np.argsort(-sel, axis=1, kind="stable")[:, :TOPK]
    topk_w = np.take_along_axis(p, topk_idx, axis=1) * ROUTED_SCALING
    return topk_idx, topk_w


def _pack_inputs(x8_toks, wg_e, wu_e, wd_e):
    """Device-layout packing for one expert: x8_toks [n<=C, H] fp8."""
    f8 = ml_dtypes.float8_e4m3

    def q8(a):
        return np.clip(a, -240.0, 240.0).astype(f8)

    n = len(x8_toks)
    xg = np.zeros((C, H), dtype=f8)
    xg[:n] = x8_toks
    xTd = np.ascontiguousarray(xg.T.reshape(HO, P, C).transpose(1, 0, 2))
    wgd = q8(SW * wg_e).reshape(HO, P, IO, P).transpose(2, 1, 0, 3)
    wud = q8(SW * wu_e).reshape(HO, P, IO, P).transpose(2, 1, 0, 3)
    wgud = np.ascontiguousarray(np.stack([wgd, wud], axis=2))  # [IO,P,2,HO,P]
    wdd = np.ascontiguousarray(
        q8(SW * wd_e).reshape(IO, P, HO, P)
        .transpose(2, 1, 0, 3)                 # [HO, P, IO, P]
        .reshape(4, 4, P, IO, P)
        .transpose(0, 2, 1, 3, 4))             # [4, P, 4, IO, P]
    return {"xT": xTd, "wgu": wgud, "wd4": wdd}


def kernel(hidden_states, router_w, corr_bias, w_gate, w_up, w_down):
    global _PROGRAM, LAST_RESULTS
    x = np.asarray(hidden_states, dtype=np.float32)
    router_w = np.asarray(router_w, dtype=np.float32)
    corr_bias = np.asarray(corr_bias, dtype=np.float32)
    w_gate = np.asarray(w_gate, dtype=np.float32)
    w_up = np.asarray(w_up, dtype=np.float32)
    w_down = np.asarray(w_down, dtype=np.float32)

    topk_idx, topk_w = _route(x, router_w, corr_bias)
    routed = topk_idx < E
    zero_w = (topk_w * (~routed)).sum(axis=1)  # [T] fp64

    f8 = ml_dtypes.float8_e4m3  # TRN fp8e4: e4m3 with max normal 240

    def q8(a):
        return np.clip(a, -240.0, 240.0).astype(f8)

    x8 = q8(x)

    # Dispatch: token list + gate weight per expert; overflow beyond C
    # falls back to an exact host computation.
    idx_list, w_list, overflow = [], [], []
    for e in range(E):
        toks, kpos = np.nonzero(topk_idx == e)
        we = topk_w[toks, kpos]
        if len(toks) > C:
            overflow.append((e, toks[C:], we[C:]))
            toks, we = toks[:C], we[:C]
        idx_list.append(toks)
        w_list.append(we)

    in_maps = [
        _pack_inputs(x8[idx_list[e]], w_gate[e], w_up[e], w_down[e])
        for e in range(E)
    ]

    if _PROGRAM is None:
        _PROGRAM = _build_program()

    from concourse.bass_utils import run_bass_kernel_spmd

    kw = {}
    if os.environ.get("MOE_KERNEL_TRACE", "") == "1":
        kw = dict(trace=True, trace_cores=list(range(N_CORES)))
    res = run_bass_kernel_spmd(
        _PROGRAM, in_maps, core_ids=list(range(N_CORES)), **kw)
    LAST_RESULTS = res

    out = np.zeros((T, H), dtype=np.float64)
    inv = 1.0 / (SW * SM)
    for e in range(E):
        n = len(idx_list[e])
        if n:
            y2e = res.results[e]["y2"]  # [HO/2, P, 2, C] bf16 = 256 * down
            ye = y2e.transpose(0, 2, 1, 3).reshape(H, C)
            out[idx_list[e]] += (w_list[e] * inv)[:, None] * \
                ye[:, :n].T.astype(np.float64)
    for e, toks, ws in overflow:
        xt = x[toks].astype(np.float64)
        g = xt @ w_gate[e].astype(np.float64)
        u = xt @ w_up[e].astype(np.float64)
        mid = (g / (1.0 + np.exp(-g))) * u
        out[toks] += ws[:, None] * (mid @ w_down[e].astype(np.float64))
    out += zero_w[:, None] * x.astype(np.float64)
    return out.astype(np.float32)
